# revision 1
# baseline (speedup 1.0000x reference)
"""DeepSet Bass kernel v3 for 8 trn2 NeuronCores.

Sharding: data-parallel over segments (16384 -> 8 x 2048), 16 windows of 128
segments per core. Host pads every segment to a multiple of 8 rows (pad rows
x=0) so pooling can run on 8:1-reduced "super-rows".

Per-core dataflow, per window (PAIRS tile-pairs of 256 rows):
  - block-diag packing: xT2[16, cols] holds TWO 128-row tiles per column
    block (rows j and j+128 of a pair stacked on partitions 0:8 / 8:16).
  - L1 (f32r): z1[128, F] = w1d[16,128].T @ xw2 - 64 feats of tile A on
    partitions 0:64, tile B on 64:128 (blockdiag W1).
  - relu1 (+b1 bias) -> h1a[128, F] bf16 (engines rotated ACT/DVE/Pool).
  - L2 (bf16): z2[128, F] = w2d(blockdiag)[128,128].T @ h1a.
  - relu2 (+b2) -> h2t[128, F] bf16.
  - reduce-8 (DVE): s8[128, F/8] f32 - sums 8 consecutive rows (all within
    one segment thanks to host padding).
  - PE-transpose s8 -> s8T[F/8, 128] (A-feats cols 0:64 | B-feats 64:128),
    copy to SBUF bf16.
  - pool (bf16): pooled[128segs, 64] += onehot[supers,128segs].T @ s8T half;
    onehots for the whole window generated in ONE DVE is_equal (bf16).
  - pad-correction: pooled += padcnt[1,128segs] x (-h2_pad) rank-1 matmul
    (h2_pad = relu(W2 relu(b1) + b2) is the phi output of an x=0 pad row,
    computed on host); makes pad rows contribute exactly zero.
  - tail per 512-seg chunk (all f32r, as v2): phi-L3 (+counts*b3) then rho.
Host gathers 8 x [4, 2048] -> [16384, 4].
"""

import os as _os
import sys

import numpy as np

sys.path.insert(0, "/opt/trn_rl_repo")

import concourse.bass as bass  # noqa: E402
import concourse.mybir as mybir  # noqa: E402
import concourse.tile as tile  # noqa: E402
from concourse import bacc  # noqa: E402
from concourse.bass_utils import run_bass_kernel_spmd  # noqa: E402
from concourse.masks import make_identity  # noqa: E402

F32 = mybir.dt.float32
F32R = mybir.dt.float32r
BF16 = mybir.dt.bfloat16
I32 = mybir.dt.int32
AF = mybir.ActivationFunctionType
ALU = mybir.AluOpType
NP_BF16 = mybir.dt.np(BF16)

NUM_SEGMENTS = 16384
N_CORES = 8
SEG_PER_CORE = NUM_SEGMENTS // N_CORES  # 2048
WIN_SEGS = 128
N_WIN = SEG_PER_CORE // WIN_SEGS  # 16
N_GWIN = NUM_SEGMENTS // WIN_SEGS  # 128
STATE_DIM = 8
HID = 64
OUT_DIM = 4
CHUNK = 512  # segs per batched rho-tail chunk (4 windows)

_BUILD_CACHE: dict[tuple, object] = {}


def _gp():
    return int(_os.environ.get("V3GP", "4"))  # max pairs per op group


def _groups(pairs: int):
    """[(pair_start, npairs)] covering `pairs` in chunks of at most GP."""
    out = []
    q = 0
    gp = _gp()
    while q < pairs:
        np_ = min(gp, pairs - q)
        out.append((q, np_))
        q += np_
    return out


def _build_program(pairs: int, reps: int = 1):
    # PSUM-reading ops (relus, psum->sbuf copies) may only use act/dve;
    # the Pool engine (gpsimd) cannot access PSUM.
    r1rot = _os.environ.get("V3R1", "act").split(",")
    r2rot = _os.environ.get("V3R2", "act").split(",")
    cprot = _os.environ.get("V3CP", "dve").split(",")
    ohrot = _os.environ.get("V3OH", "dve").split(",")
    pd = _os.environ.get("V3PD", "bf16")
    s8d = _os.environ.get("V3S8", "f32")  # s8/transpose dtype: f32 | bf16
    wb = int(_os.environ.get("V3WB", "3"))
    xb = int(_os.environ.get("V3XB", "2"))
    GP = _gp()
    pb = _os.environ.get("V3PB", "a")  # PSUM buf split: a=stps2/pool1, b=stps1/pool2
    key = (pairs, reps, ",".join(r1rot), ",".join(r2rot), ",".join(cprot),
           ",".join(ohrot), pd, s8d, wb, xb, GP, pb)
    if key in _BUILD_CACHE:
        return _BUILD_CACHE[key]

    groups = _groups(pairs)
    NH = 2 * len(groups)  # onehot halves per window
    WCOLS = pairs * 128  # xT2 cols per window
    NSUP = pairs * 32  # supers per window
    PDT = BF16 if pd == "bf16" else F32
    S8D = BF16 if s8d == "bf16" else F32

    nc = bacc.Bacc("TRN2", target_bir_lowering=False, debug=False, num_devices=N_CORES)

    xT_d = nc.declare_dram_parameter("xT", [16, N_WIN * WCOLS], F32R, isOutput=False)
    ids_d = nc.declare_dram_parameter("ids", [64, N_WIN * NH], PDT, isOutput=False)
    cnt_d = nc.declare_dram_parameter("cnt", [1, SEG_PER_CORE], F32R, isOutput=False)
    pad_d = nc.declare_dram_parameter("pad", [1, SEG_PER_CORE], F32R, isOutput=False)
    w1d_d = nc.declare_dram_parameter("w1d", [16, 128], F32R, isOutput=False)
    b1d_d = nc.declare_dram_parameter("b1d", [128, 1], F32, isOutput=False)
    w2d_d = nc.declare_dram_parameter("w2d", [128, 128], BF16, isOutput=False)
    b2d_d = nc.declare_dram_parameter("b2d", [128, 1], F32, isOutput=False)
    nh2p_d = nc.declare_dram_parameter("nh2p", [1, HID], F32R, isOutput=False)
    w3a_d = nc.declare_dram_parameter("w3a", [HID + 1, HID], F32R, isOutput=False)
    rw1_d = nc.declare_dram_parameter("rw1", [HID, HID], F32R, isOutput=False)
    rw2_d = nc.declare_dram_parameter("rw2", [HID, HID], F32R, isOutput=False)
    rw3_d = nc.declare_dram_parameter("rw3", [HID, OUT_DIM], F32R, isOutput=False)
    rb1_d = nc.declare_dram_parameter("rb1", [HID, 1], F32, isOutput=False)
    rb2_d = nc.declare_dram_parameter("rb2", [HID, 1], F32, isOutput=False)
    rb3_d = nc.declare_dram_parameter("rb3", [OUT_DIM, 1], F32, isOutput=False)
    out_d = nc.declare_dram_parameter("out", [OUT_DIM, SEG_PER_CORE], F32, isOutput=True)

    ENG = {"act": None, "dve": None, "pool": None}  # filled after nc exists

    with tile.TileContext(nc) as tc:
        with (
            tc.tile_pool(name="const", bufs=1) as cpool,
            tc.tile_pool(name="xwin", bufs=xb) as xpool,
            tc.tile_pool(name="work", bufs=wb) as wpool,
            tc.tile_pool(name="oh", bufs=2) as ohpool,
            tc.tile_pool(name="chunk", bufs=2) as chpool,
            tc.tile_pool(name="z1ps", bufs=1 if GP >= 8 else 2, space="PSUM") as z1ps,
            tc.tile_pool(name="z2ps", bufs=1, space="PSUM") as z2ps,
            tc.tile_pool(
                name="stps", bufs=1 if (pb == "b" or GP >= 8) else 2, space="PSUM"
            ) as stps,
            tc.tile_pool(
                name="poolps", bufs=2 if pb == "b" else 1, space="PSUM"
            ) as poolps,
            tc.tile_pool(name="tailps", bufs=2, space="PSUM") as tailps,
        ):
            ENG = {"act": nc.scalar, "dve": nc.vector, "pool": nc.gpsimd}

            def cload(name, shape, dram, dt=F32):
                t = cpool.tile(shape, dt, tag=name)
                nc.sync.dma_start(out=t[:], in_=dram[:])
                return t

            w1d = cload("w1d", [16, 128], w1d_d, F32R)
            b1d = cload("b1d", [128, 1], b1d_d)
            w2d = cload("w2d", [128, 128], w2d_d, BF16)
            b2d = cload("b2d", [128, 1], b2d_d)
            nh2p = cload("nh2p", [1, HID], nh2p_d, F32R)
            padw = cload("pad", [1, SEG_PER_CORE], pad_d, F32R)
            idrS = cload("ids", [64, N_WIN * NH], ids_d, PDT)
            w3a = cload("w3a", [HID + 1, HID], w3a_d, F32R)
            rw1 = cload("rw1", [HID, HID], rw1_d, F32R)
            rw2 = cload("rw2", [HID, HID], rw2_d, F32R)
            rw3 = cload("rw3", [HID, OUT_DIM], rw3_d, F32R)
            rb1 = cload("rb1", [HID, 1], rb1_d)
            rb2 = cload("rb2", [HID, 1], rb2_d)
            rb3 = cload("rb3", [OUT_DIM, 1], rb3_d)

            ident = cpool.tile([128, 128], F32, tag="ident")
            make_identity(nc, ident[:])
            if S8D == BF16:
                identB = cpool.tile([128, 128], BF16, tag="identB")
                nc.vector.tensor_copy(out=identB[:], in_=ident[:])
                ident = identB
            iota_i = cpool.tile([64, NH * 128], I32, tag="iota_i")
            nc.gpsimd.iota(
                iota_i[:], pattern=[[0, NH], [1, 128]], base=0, channel_multiplier=0
            )
            iota_f = cpool.tile([64, NH * 128], F32, tag="iota_f")
            nc.vector.tensor_copy(out=iota_f[:], in_=iota_i[:])
            if PDT == F32:
                iotaW = iota_f
            else:
                iotaW = cpool.tile([64, NH * 128], PDT, tag="iotaW")
                nc.vector.tensor_copy(out=iotaW[:], in_=iota_f[:])

            def relu(eng, out_ap, in_ap, bias_ap):
                if eng == "act":
                    nc.scalar.activation(out=out_ap, in_=in_ap, func=AF.Relu, bias=bias_ap)
                else:
                    ENG[eng].tensor_scalar(
                        out=out_ap, in0=in_ap, scalar1=bias_ap, scalar2=0.0,
                        op0=ALU.add, op1=ALU.max,
                    )

            def copy(eng, out_ap, in_ap):
                if eng == "act":
                    nc.scalar.copy(out_ap, in_ap)
                else:
                    ENG[eng].tensor_copy(out=out_ap, in_=in_ap)

            ri = 0
            for _rep in range(reps):
             for ch in range(SEG_PER_CORE // CHUNK):
                poolT = chpool.tile([HID + 1, CHUNK], F32R, tag="poolT")
                nc.sync.dma_start(
                    out=poolT[HID : HID + 1, :],
                    in_=cnt_d[:, ch * CHUNK : (ch + 1) * CHUNK],
                )
                for wl in range(CHUNK // WIN_SEGS):
                    w = ch * (CHUNK // WIN_SEGS) + wl
                    xw = xpool.tile([16, WCOLS], F32R, tag="xw")
                    nc.sync.dma_start(
                        out=xw[:], in_=xT_d[:, w * WCOLS : (w + 1) * WCOLS]
                    )
                    ohW = ohpool.tile([64, NH * 128], PDT, tag="ohW")
                    ENG[ohrot[w % len(ohrot)]].tensor_tensor(
                        out=ohW[:].rearrange("p (a b) -> p a b", b=128),
                        in0=idrS[:, w * NH : (w + 1) * NH].to_broadcast([64, NH, 128]),
                        in1=iotaW[:].rearrange("p (a b) -> p a b", b=128),
                        op=ALU.is_equal,
                    )
                    pooled_ps = poolps.tile([WIN_SEGS, HID], F32, tag="pool")

                    for g, (q0, np_) in enumerate(groups):
                        F = np_ * 128
                        NS = np_ * 16
                        z1 = z1ps.tile([128, GP * 128], F32, tag="z1")
                        nc.tensor.matmul(
                            out=z1[:, :F], lhsT=w1d[:],
                            rhs=xw[:, q0 * 128 : q0 * 128 + F],
                            start=True, stop=True,
                        )
                        h1a = wpool.tile([128, GP * 128], BF16, tag="h1a")
                        relu(r1rot[ri % len(r1rot)], h1a[:, :F], z1[:, :F], b1d[:])
                        z2 = z2ps.tile([128, GP * 128], F32, tag="z2")
                        nc.tensor.matmul(
                            out=z2[:, :F], lhsT=w2d[:], rhs=h1a[:, :F],
                            start=True, stop=True,
                        )
                        h2t = wpool.tile([128, GP * 128], BF16, tag="h2t")
                        relu(r2rot[ri % len(r2rot)], h2t[:, :F], z2[:, :F], b2d[:])
                        s8 = wpool.tile([128, GP * 16], S8D, tag="s8")
                        with nc.allow_low_precision(reason="8-elem bf16 sums"):
                            nc.vector.tensor_reduce(
                                out=s8[:, :NS],
                                in_=h2t[:, :F].rearrange("p (u e) -> p u e", e=8),
                                axis=mybir.AxisListType.X, op=ALU.add,
                            )
                        s8T_ps = stps.tile([GP * 16, 128], S8D, tag="s8T")
                        nc.tensor.transpose(
                            out=s8T_ps[:NS, :], in_=s8[:, :NS], identity=ident[:]
                        )
                        s8T = wpool.tile([GP * 16, 128], PDT, tag="s8Tc")
                        copy(cprot[ri % len(cprot)], s8T[:NS, :], s8T_ps[:NS, :])
                        for hh in range(2):
                            nc.tensor.matmul(
                                out=pooled_ps[:],
                                lhsT=ohW[:NS, (2 * g + hh) * 128 : (2 * g + hh + 1) * 128],
                                rhs=s8T[:NS, hh * HID : (hh + 1) * HID],
                                start=(g == 0 and hh == 0), stop=False,
                            )
                        ri += 1

                    # pad-row correction closes the accumulation group
                    nc.tensor.matmul(
                        out=pooled_ps[:],
                        lhsT=padw[:, w * WIN_SEGS : (w + 1) * WIN_SEGS],
                        rhs=nh2p[:], start=False, stop=True,
                    )

                    pooled_sb = wpool.tile([WIN_SEGS, HID], S8D, tag="pooled")
                    nc.scalar.activation(
                        out=pooled_sb[:], in_=pooled_ps[:], func=AF.Copy, bias=0.0
                    )
                    poolT_ps = stps.tile([GP * 16, 128], S8D, tag="s8T")
                    nc.tensor.transpose(
                        out=poolT_ps[:HID, :], in_=pooled_sb[:], identity=ident[:]
                    )
                    nc.vector.tensor_copy(
                        out=poolT[:HID, wl * WIN_SEGS : (wl + 1) * WIN_SEGS],
                        in_=poolT_ps[:HID, :],
                    )

                # batched phi-L3 + rho tail over this 512-seg chunk (f32r)
                p3_ps = tailps.tile([HID, CHUNK], F32, tag="tail")
                nc.tensor.matmul(
                    out=p3_ps[:], lhsT=w3a[:], rhs=poolT[:], start=True, stop=True
                )
                p3 = chpool.tile([HID, CHUNK], F32R, tag="p3")
                nc.scalar.activation(out=p3[:], in_=p3_ps[:], func=AF.Copy, bias=0.0)

                r1_ps = tailps.tile([HID, CHUNK], F32, tag="tail")
                nc.tensor.matmul(
                    out=r1_ps[:], lhsT=rw1[:], rhs=p3[:], start=True, stop=True
                )
                r1 = chpool.tile([HID, CHUNK], F32R, tag="r1")
                nc.scalar.activation(out=r1[:], in_=r1_ps[:], func=AF.Relu, bias=rb1[:])

                r2_ps = tailps.tile([HID, CHUNK], F32, tag="tail")
                nc.tensor.matmul(
                    out=r2_ps[:], lhsT=rw2[:], rhs=r1[:], start=True, stop=True
                )
                r2 = chpool.tile([HID, CHUNK], F32R, tag="r2")
                nc.scalar.activation(out=r2[:], in_=r2_ps[:], func=AF.Relu, bias=rb2[:])

                r3_ps = tailps.tile([OUT_DIM, CHUNK], F32, tag="tail")
                nc.tensor.matmul(
                    out=r3_ps[:], lhsT=rw3[:], rhs=r2[:], start=True, stop=True
                )
                out_sb = chpool.tile([OUT_DIM, CHUNK], F32, tag="outc")
                nc.scalar.activation(
                    out=out_sb[:], in_=r3_ps[:], func=AF.Identity, bias=rb3[:]
                )
                nc.sync.dma_start(
                    out=out_d[:, ch * CHUNK : (ch + 1) * CHUNK], in_=out_sb[:]
                )

    nc.compile()
    _BUILD_CACHE[key] = nc
    return nc


def _prep_inputs(neighbors: np.ndarray, segment_ids: np.ndarray):
    """Segment-mult-of-8 padded, block-diag-paired host marshalling."""
    x = np.asarray(neighbors, dtype=np.float32)
    ids = np.asarray(segment_ids, dtype=np.int64)
    total = ids.shape[0]

    cnt_s = np.bincount(ids, minlength=NUM_SEGMENTS)  # real rows per seg
    len8 = (cnt_s + 7) // 8 * 8  # padded rows per seg
    win_of_seg = np.arange(NUM_SEGMENTS) // WIN_SEGS

    # per-window padded row counts and per-seg offsets within the window
    rows8_w = np.zeros(N_GWIN, dtype=np.int64)
    np.add.at(rows8_w, win_of_seg, len8)
    c8 = np.cumsum(len8)
    off8 = c8 - len8  # global cumulative
    win_base8 = np.zeros(N_GWIN, dtype=np.int64)
    win_base8[1:] = np.cumsum(rows8_w)[:-1]
    off8_in_win = off8 - win_base8[win_of_seg]  # seg start within window

    pairs = int(np.ceil(rows8_w.max() / 256))
    PW2 = pairs * 256

    # scatter rows into the padded per-window layout
    seg_start = np.cumsum(cnt_s) - cnt_s
    idx_in_seg = np.arange(total) - seg_start[ids]
    dst = off8_in_win[ids] + idx_in_seg  # row slot within window
    win_of_row = win_of_seg[ids]
    xpad = np.zeros((N_GWIN, PW2, STATE_DIM), dtype=np.float32)
    xpad[win_of_row, dst] = x

    # block-diag pair layout: [win, pairs, 2, 128, 8] -> [win, 16, pairs*128]
    xp = xpad.reshape(N_GWIN, pairs, 2, 128, STATE_DIM)
    xT2 = (
        xp.transpose(0, 1, 2, 4, 3)  # win, pair, half, 8, 128
        .reshape(N_GWIN, pairs, 16, 128)
        .transpose(0, 2, 1, 3)  # win, 16, pair, 128
        .reshape(N_GWIN, 16, pairs * 128)
    )

    # supers: seg (window-local) owning each 8-row block; tail-pad supers -> 0
    nsup = PW2 // 8
    sup_seg = np.zeros((N_GWIN, nsup), dtype=np.int64)
    seg_local = np.arange(NUM_SEGMENTS) % WIN_SEGS
    sup_vals = np.repeat(seg_local, len8 // 8)  # all real supers, window-major
    sup_w = np.repeat(win_of_seg, len8 // 8)
    sup_off = np.zeros(N_GWIN, dtype=np.int64)
    np.add.at(sup_off, sup_w, 0)  # noop, placeholder
    # position within window: cumulative super index
    csup = np.cumsum(len8 // 8)
    sup_start = (csup - len8 // 8) - (win_base8[win_of_seg] // 8)
    pos = np.repeat(sup_start, len8 // 8) + (
        np.arange(len(sup_vals))
        - np.repeat(csup - len8 // 8, len8 // 8)
    )
    sup_seg[sup_w, pos] = sup_vals

    # idrS in transpose order: [win, pair, half(2), 16] -> halves per group
    groups = _groups(pairs)
    NH = 2 * len(groups)
    sup_q = sup_seg.reshape(N_GWIN, pairs, 2, 16)
    idrS = np.zeros((N_GWIN, 64, NH), dtype=np.float32)
    for g, (q0, np_) in enumerate(groups):
        for hh in range(2):
            v = sup_q[:, q0 : q0 + np_, hh, :].reshape(N_GWIN, np_ * 16)
            idrS[:, : np_ * 16, 2 * g + hh] = v

    # pad counts: seg-align pads to own seg, window tail pads to local seg 0
    padcnt = (len8 - cnt_s).astype(np.float32).reshape(N_GWIN, WIN_SEGS)
    padcnt[:, 0] += (PW2 - rows8_w).astype(np.float32)

    cnt = cnt_s.astype(np.float32).reshape(N_GWIN, WIN_SEGS)
    return xT2, idrS, padcnt, cnt, pairs


def prep_maps(inputs: dict):
    xT2, idrS, padcnt, cnt, pairs = _prep_inputs(
        inputs["neighbors"], inputs["segment_ids"]
    )
    f = lambda a: np.ascontiguousarray(np.asarray(a, dtype=np.float32))
    col = lambda a: f(a).reshape(-1, 1)
    W1, b1 = f(inputs["phi_W1"]), f(inputs["phi_b1"])
    W2, b2 = f(inputs["phi_W2"]), f(inputs["phi_b2"])
    w1d = np.zeros((16, 128), np.float32)
    w1d[0:8, 0:64] = W1
    w1d[8:16, 64:128] = W1
    b1d = np.concatenate([col(b1), col(b1)], axis=0)
    w2d = np.zeros((128, 128), np.float32)
    w2d[0:64, 0:64] = W2
    w2d[64:128, 64:128] = W2
    b2d = np.concatenate([col(b2), col(b2)], axis=0)
    # phi output of an x=0 pad row
    h1p = np.maximum(b1, 0.0)
    h2p = np.maximum(h1p @ W2 + b2, 0.0)
    nh2p = -h2p.reshape(1, HID).astype(np.float32)

    w3a = np.vstack([f(inputs["phi_W3"]), f(inputs["phi_b3"]).reshape(1, -1)])
    pd_np = NP_BF16 if _os.environ.get("V3PD", "bf16") == "bf16" else np.float32
    shared = {
        "w1d": w1d, "b1d": b1d, "w2d": w2d.astype(NP_BF16), "b2d": b2d,
        "nh2p": nh2p, "w3a": w3a,
        "rw1": f(inputs["rho_W1"]), "rw2": f(inputs["rho_W2"]),
        "rw3": f(inputs["rho_W3"]),
        "rb1": col(inputs["rho_b1"]), "rb2": col(inputs["rho_b2"]),
        "rb3": col(inputs["rho_b3"]),
    }
    in_maps = []
    for c in range(N_CORES):
        ws = slice(c * N_WIN, (c + 1) * N_WIN)
        in_maps.append(
            {
                "xT": np.ascontiguousarray(
                    xT2[ws].transpose(1, 0, 2).reshape(16, -1)
                ),
                "ids": np.ascontiguousarray(
                    idrS[ws].transpose(1, 0, 2).reshape(64, -1)
                ).astype(pd_np),
                "cnt": cnt[ws].reshape(1, -1),
                "pad": padcnt[ws].reshape(1, -1),
                **shared,
            }
        )
    return pairs, in_maps


def kernel(**inputs):
    pairs, in_maps = prep_maps(inputs)
    nc = _build_program(pairs)
    res = run_bass_kernel_spmd(nc, in_maps, core_ids=list(range(N_CORES)))
    out = np.concatenate(
        [res.results[c]["out"].T for c in range(N_CORES)], axis=0
    ).astype(np.float32)
    return out

